# revision 1
# baseline (speedup 1.0000x reference)
"""DecoderLSTM Trainium2 kernel.

Data-parallel over batch: B=512 is sharded 64-per-core across 8 NeuronCores;
LSTM/FC weights are replicated and streamed from HBM each step (63 MB fp32
exceeds the 24 MB SBUF).  All matmuls run in fp32: the 96-step recurrence is
chaotic (measured error amplification ~250x), so reduced-precision matmuls
diverge (bf16 -> 60% rel err, fp32r -> 2.4%); fp32 lands ~1e-4.

Per-core layout:
  Big matmuls ("orientation A"): out[b, feat] accumulates in PSUM, lhsT =
  transposed activations [K, 64] stationary, rhs = streamed weight tiles
  [K, 512] moving.  Gate biases are added during PSUM evacuation on DVE.
  Small matmuls (embedding, fc2) run "orientation B" (weights stationary),
  producing transposed outputs directly — y feeds back as x with no
  transpose.  h0/h1/relu transposes use the PE transpose path.

Self-contained: shapes hardcoded; nothing read from the problem directory.
"""
from contextlib import ExitStack

import numpy as np

import concourse.bass as bass
import concourse.tile as tile
from concourse import bacc, mybir
from concourse import bass_utils

F32 = mybir.dt.float32
AF = mybir.ActivationFunctionType
ALU = mybir.AluOpType

B, D, E, H, T_FULL = 512, 64, 512, 1024, 96
NC = 8
BC = B // NC          # 64 batch rows per core
KC_E = E // 128       # 4
KC_H = H // 128       # 8
G4H = 4 * H           # 4096
LN_EPS = 1e-5

_cache = {}


def _emit(ctx: ExitStack, tc: tile.TileContext, io: dict, t_steps: int):
    nc = tc.nc

    res = ctx.enter_context(tc.tile_pool(name="resident", bufs=1))
    state = ctx.enter_context(tc.tile_pool(name="state", bufs=2))
    work = ctx.enter_context(tc.tile_pool(name="work", bufs=1))
    wstream = ctx.enter_context(tc.tile_pool(name="wstream", bufs=3))
    gpsum = ctx.enter_context(tc.tile_pool(name="gpsum", bufs=4, space="PSUM"))
    tpsum = ctx.enter_context(tc.tile_pool(name="tpsum", bufs=2, space="PSUM"))
    spsum = ctx.enter_context(tc.tile_pool(name="spsum", bufs=2, space="PSUM"))

    # ---- resident tensors (loaded once) ----
    emb_W = res.tile([64, E], F32)               # [D, E]; lhsT chunks [:, c*128:]
    fc2_W = res.tile([128, KC_H, 64], F32)       # fc2 lhsT chunks
    ident = res.tile([64, 64], F32)
    b0_bc = res.tile([BC, G4H], F32)             # gate biases bcast over batch
    b1_bc = res.tile([BC, G4H], F32)
    fc1_b_bc = res.tile([BC, H], F32)
    emb_bT = res.tile([128, KC_E], F32)          # per-partition bias, chunk c
    fc2_bT = res.tile([64, 1], F32)
    g_bc = res.tile([BC, H], F32)
    bb_bc = res.tile([BC, H], F32)

    for name, t in [("emb_W", emb_W), ("ident", ident), ("b0_bc", b0_bc),
                    ("b1_bc", b1_bc), ("fc1_b_bc", fc1_b_bc),
                    ("emb_bT", emb_bT), ("fc2_bT", fc2_bT), ("g_bc", g_bc),
                    ("bb_bc", bb_bc)]:
        nc.sync.dma_start(t[:], io[name].ap())
    nc.sync.dma_start(fc2_W[:], io["fc2_W"].ap().rearrange("(k p) o -> p k o", p=128))

    # ---- initial state ----
    xT0 = res.tile([64, BC], F32)
    nc.sync.dma_start(xT0[:], io["xT0"].ap())
    eps_t = res.tile([BC, 1], F32)
    nc.vector.memset(eps_t[:], LN_EPS)
    h0T = state.tile([128, KC_H, BC], F32, tag="h0T")
    h1T = state.tile([128, KC_H, BC], F32, tag="h1T")
    c0 = state.tile([BC, H], F32, tag="c0")
    c1 = state.tile([BC, H], F32, tag="c1")
    nc.sync.dma_start(h0T[:], io["h0T0"].ap().rearrange("(k p) b -> p k b", p=128))
    nc.sync.dma_start(h1T[:], io["h1T0"].ap().rearrange("(k p) b -> p k b", p=128))
    nc.sync.dma_start(c0[:], io["c00"].ap())
    nc.sync.dma_start(c1[:], io["c10"].ap())

    y_last = None

    def lstm_layer(layer, xe_lhsT, hT_prev, c_prev, w_in_dram, w_hh_dram,
                   b_bc, kc_in):
        """Gates + cell update.  Returns (h_new [BC,H] sbuf, c_new)."""
        gts = work.tile([BC, G4H], F32, tag=f"gts{layer}")
        for half in range(2):
            gb = [gpsum.tile([BC, 512], F32, tag="gb", name=f"gb{half}_{_n}")
                  for _n in range(4)]
            # recurrent part first (hT_prev ready since last step)
            for k in range(KC_H):
                wt = wstream.tile([128, 2048], F32, tag="wstream")
                nc.sync.dma_start(
                    wt[:], w_hh_dram.ap()[k * 128:(k + 1) * 128,
                                          half * 2048:(half + 1) * 2048])
                for n in range(4):
                    nc.tensor.matmul(gb[n][:], hT_prev[:, k, :],
                                     wt[:, n * 512:(n + 1) * 512],
                                     start=(k == 0), stop=False)
            # input part
            for k in range(kc_in):
                wt = wstream.tile([128, 2048], F32, tag="wstream")
                nc.sync.dma_start(
                    wt[:], w_in_dram.ap()[k * 128:(k + 1) * 128,
                                          half * 2048:(half + 1) * 2048])
                lhsT = xe_lhsT(k)
                for n in range(4):
                    nc.tensor.matmul(gb[n][:], lhsT,
                                     wt[:, n * 512:(n + 1) * 512],
                                     start=False, stop=(k == kc_in - 1))
            # evacuate with bias add (DVE), then in-place nonlinearity (ACT)
            for n in range(4):
                col = half * 2048 + n * 512
                nc.vector.tensor_add(gts[:, col:col + 512], gb[n][:],
                                     b_bc[:, col:col + 512])
        # i f g o, each H wide
        nc.scalar.activation(gts[:, 0:2 * H], gts[:, 0:2 * H], AF.Sigmoid)
        nc.scalar.activation(gts[:, 2 * H:3 * H], gts[:, 2 * H:3 * H], AF.Tanh)
        nc.scalar.activation(gts[:, 3 * H:], gts[:, 3 * H:], AF.Sigmoid)

        c_new = state.tile([BC, H], F32, tag=f"c{layer}")
        tmp1 = work.tile([BC, H], F32, tag="tmp1")
        tanh_c = work.tile([BC, H], F32, tag=f"tanh_c{layer}")
        h_new = work.tile([BC, H], F32, tag=f"h{layer}")
        nc.vector.tensor_mul(tmp1[:], gts[:, H:2 * H], c_prev[:])
        nc.vector.tensor_mul(c_new[:], gts[:, 0:H], gts[:, 2 * H:3 * H])
        nc.vector.tensor_add(c_new[:], c_new[:], tmp1[:])
        nc.scalar.activation(tanh_c[:], c_new[:], AF.Tanh)
        nc.vector.tensor_mul(h_new[:], gts[:, 3 * H:], tanh_c[:])
        return h_new, c_new

    def transpose_to(hT_new, h_sb):
        """h [BC, H] -> hT [128, KC_H, BC] via PE transposes."""
        for ck in range(KC_H):
            tp = tpsum.tile([128, BC], F32, tag="tp")
            nc.tensor.transpose(tp[:], h_sb[:, ck * 128:(ck + 1) * 128],
                                ident[:])
            nc.vector.tensor_copy(hT_new[:, ck, :], tp[:])

    for t in range(t_steps):
        xT = xT0[:] if t == 0 else y_last[:]

        # ---- embedding (orientation B): xeT[c] = emb_W[:,c].T @ xT ----
        xeT = work.tile([128, KC_E, BC], F32, tag="xeT")
        for c in range(KC_E):
            xp = spsum.tile([128, BC], F32, tag="sp")
            nc.tensor.matmul(xp[:], emb_W[:, c * 128:(c + 1) * 128], xT,
                             start=True, stop=True)
            nc.vector.tensor_scalar_add(xeT[:, c, :], xp[:], emb_bT[:, c:c + 1])

        # ---- LSTM layers ----
        h0_new, c0_new = lstm_layer(
            0, lambda k: xeT[:, k, :], h0T, c0,
            io["W_ih0"], io["W_hh0"], b0_bc, KC_E)
        h0T_new = state.tile([128, KC_H, BC], F32, tag="h0T")
        transpose_to(h0T_new, h0_new)

        h1_new, c1_new = lstm_layer(
            1, lambda k: h0T_new[:, k, :], h1T, c1,
            io["W_ih1"], io["W_hh1"], b1_bc, KC_H)
        h1T_new = state.tile([128, KC_H, BC], F32, tag="h1T")
        transpose_to(h1T_new, h1_new)

        # ---- fc1 + LayerNorm + ReLU ----
        z = work.tile([BC, H], F32, tag="z")
        z_sums = work.tile([BC, 2], F32, tag="z_sums")
        zp = [spsum.tile([BC, 512], F32, tag="sp", name=f"zp{_n}")
               for _n in range(2)]
        for k in range(KC_H):
            wt = wstream.tile([128, H], F32, tag="wstream")
            nc.sync.dma_start(wt[:],
                              io["fc1_W"].ap()[k * 128:(k + 1) * 128, :])
            for n in range(2):
                nc.tensor.matmul(zp[n][:], h1T_new[:, k, :],
                                 wt[:, n * 512:(n + 1) * 512],
                                 start=(k == 0), stop=(k == KC_H - 1))
        for n in range(2):
            nc.vector.tensor_add(z[:, n * 512:(n + 1) * 512], zp[n][:],
                                 fc1_b_bc[:, n * 512:(n + 1) * 512])
            nc.vector.reduce_sum(z_sums[:, n:n + 1],
                                 z[:, n * 512:(n + 1) * 512],
                                 axis=mybir.AxisListType.X)
        mu = work.tile([BC, 1], F32, tag="mu")
        negmu = work.tile([BC, 1], F32, tag="negmu")
        sqs = work.tile([BC, 1], F32, tag="sqs")
        sq = work.tile([BC, H], F32, tag="sq")
        va = work.tile([BC, 1], F32, tag="va")
        sv = work.tile([BC, 1], F32, tag="sv")
        rstd = work.tile([BC, 1], F32, tag="rstd")
        nc.vector.tensor_add(mu[:], z_sums[:, 0:1], z_sums[:, 1:2])
        nc.vector.tensor_scalar_mul(negmu[:], mu[:], -1.0 / H)
        nc.vector.tensor_scalar_mul(mu[:], mu[:], 1.0 / H)
        nc.scalar.activation(sq[:], z[:], AF.Square, bias=negmu[:],
                             accum_out=sqs[:])
        nc.vector.tensor_scalar_mul(va[:], sqs[:], 1.0 / H)
        nc.scalar.activation(sv[:], va[:], AF.Sqrt, bias=eps_t[:])
        nc.vector.reciprocal(rstd[:], sv[:])
        zn = work.tile([BC, H], F32, tag="zn")
        nc.vector.tensor_scalar(zn[:], z[:], mu[:], rstd[:],
                                ALU.subtract, ALU.mult)
        nc.vector.tensor_mul(zn[:], zn[:], g_bc[:])
        nc.vector.tensor_add(zn[:], zn[:], bb_bc[:])
        nc.scalar.activation(zn[:], zn[:], AF.Relu)

        reluT = work.tile([128, KC_H, BC], F32, tag="reluT")
        transpose_to(reluT, zn)

        # ---- fc2 (orientation B): yT = fc2_W.T @ reluT ----
        yp = spsum.tile([64, BC], F32, tag="sp")
        for k in range(KC_H):
            nc.tensor.matmul(yp[:], fc2_W[:, k, :], reluT[:, k, :],
                             start=(k == 0), stop=(k == KC_H - 1))
        y_new = state.tile([64, BC], F32, tag="ylast")
        nc.vector.tensor_scalar_add(y_new[:], yp[:], fc2_bT[:])
        nc.sync.dma_start(io["ysT"].ap()[:, t, :], y_new[:])

        h0T, h1T, c0, c1, y_last = h0T_new, h1T_new, c0_new, c1_new, y_new


def build(t_steps=T_FULL):
    if t_steps in _cache:
        return _cache[t_steps]
    nc = bacc.Bacc("TRN2", target_bir_lowering=False, debug=False)
    io = {}
    inputs = [
        ("xT0", (64, BC)), ("h0T0", (H, BC)), ("h1T0", (H, BC)),
        ("c00", (BC, H)), ("c10", (BC, H)),
        ("W_ih0", (E, G4H)), ("W_hh0", (H, G4H)),
        ("W_ih1", (H, G4H)), ("W_hh1", (H, G4H)),
        ("fc1_W", (H, H)), ("fc2_W", (H, 64)), ("emb_W", (64, E)),
        ("b0_bc", (BC, G4H)), ("b1_bc", (BC, G4H)),
        ("fc1_b_bc", (BC, H)), ("emb_bT", (128, KC_E)),
        ("fc2_bT", (64, 1)), ("g_bc", (BC, H)), ("bb_bc", (BC, H)),
        ("ident", (64, 64)),
    ]
    for name, shape in inputs:
        io[name] = nc.dram_tensor(name, shape, F32, kind="ExternalInput")
    io["ysT"] = nc.dram_tensor("ysT", (64, t_steps, BC), F32,
                               kind="ExternalOutput")
    with tile.TileContext(nc) as tc:
        with ExitStack() as ctx:
            _emit(ctx, tc, io, t_steps)
    nc.compile()
    _cache[t_steps] = (nc, io)
    return nc, io


def make_in_maps(inputs):
    """Shard + transform full inputs into 8 per-core input maps."""
    f = lambda x: np.ascontiguousarray(np.asarray(x), dtype=np.float32)
    x0 = f(inputs["x_0"])
    hn = f(inputs["h_n"])
    cn = f(inputs["c_n"])
    base = {
        "W_ih0": f(inputs["W_ih0"]), "W_hh0": f(inputs["W_hh0"]),
        "W_ih1": f(inputs["W_ih1"]), "W_hh1": f(inputs["W_hh1"]),
        "fc1_W": f(inputs["fc1_W"]), "fc2_W": f(inputs["fc2_W"]),
        "emb_W": f(inputs["emb_W"]),
        "b0_bc": np.tile((f(inputs["b_ih0"]) + f(inputs["b_hh0"]))[None, :],
                         (BC, 1)),
        "b1_bc": np.tile((f(inputs["b_ih1"]) + f(inputs["b_hh1"]))[None, :],
                         (BC, 1)),
        "fc1_b_bc": np.tile(f(inputs["fc1_b"])[None, :], (BC, 1)),
        "emb_bT": np.ascontiguousarray(f(inputs["emb_b"]).reshape(KC_E, 128).T),
        "fc2_bT": f(inputs["fc2_b"])[:, None],
        "g_bc": np.tile(f(inputs["ln_g"])[None, :], (BC, 1)),
        "bb_bc": np.tile(f(inputs["ln_b"])[None, :], (BC, 1)),
        "ident": np.eye(64, dtype=np.float32),
    }
    in_maps = []
    for c in range(NC):
        sl = slice(c * BC, (c + 1) * BC)
        m = dict(base)
        m["xT0"] = np.ascontiguousarray(x0[sl].T)
        m["h0T0"] = np.ascontiguousarray(hn[0, sl].T)
        m["h1T0"] = np.ascontiguousarray(hn[1, sl].T)
        m["c00"] = np.ascontiguousarray(cn[0, sl])
        m["c10"] = np.ascontiguousarray(cn[1, sl])
        in_maps.append(m)
    return in_maps


def kernel(**inputs):
    t_steps = int(inputs.get("forecast_window", T_FULL))
    nc, io = build(t_steps)
    in_maps = make_in_maps(inputs)
    r = bass_utils.run_bass_kernel_spmd(nc, in_maps, core_ids=list(range(NC)))
    out = np.empty((B, t_steps, D), np.float32)
    for c in range(NC):
        ysT = r.results[c]["ysT"]              # [D, t, BC]
        out[c * BC:(c + 1) * BC] = ysT.transpose(2, 1, 0)
    return out



# revision 9
# speedup vs baseline: 1.4342x; 1.4342x over previous
"""DecoderLSTM Trainium2 kernel — tensor-parallel over gate columns.

Topology: 8 NeuronCores as 2 quads x 4 members (TP4 x DP2).
  - Quad q handles batch rows [256q, 256q+256); all 4 members share them.
  - Member m owns gate columns {g*1024 + [256m,256m+256) : g in i,f,g,o},
    i.e. hidden slice hm = [256m, 256m+256) of both LSTM layers, and rows
    hm of fc1 (K-sharded fc1 -> AllReduce of z partials).
  - All weights are SBUF-resident as fp16 hi/lo pairs (~14 MB/core); the
    embedding is folded into layer-0 input weights on the host in float64
    (xe @ W_ih0 == y @ (emb_W @ W_ih0)), with the layer-0 bias folded in
    as a 65th input row against a constant-one activation row.

Matmuls run as 3-pass fp16 (hi*hi + lo*hi + hi*lo, fp32 PSUM accumulate):
measured 3.6e-7 max rel err per matmul (fp32-level) at 3 cycles/row vs
fp32's 4.  States c0/c1 stay fp32 and local; h0/h1 cross cores as fp16
hi/lo pairs via AllGather; z crosses as fp32 via AllReduce.

Per-step comm (DRAM bounce collectives): AG(h0T pair 256KB), AR(z 1MB),
AG(h1T pair 256KB), software-pipelined so next-step gate matmuls cover
collective latency.

Self-contained: shapes/sharding hardcoded; reads nothing from disk.
"""
from contextlib import ExitStack

import numpy as np

import concourse.bass as bass
import concourse.tile as tile
from concourse import bacc, mybir
from concourse import bass_utils

F32 = mybir.dt.float32
F16 = mybir.dt.float16
AF = mybir.ActivationFunctionType
ALU = mybir.AluOpType

B, D, H, T_FULL = 512, 64, 1024, 96
NC = 8
B2 = 256          # batch rows per quad
MC = 2            # 128-row chunks of B2
HS = 256          # hidden shard per member
G = 1024          # gate columns per member (4 * HS)
KT = H // 128     # 8 k-tiles over H
LN_EPS = 1e-5
GROUPS = [[0, 1, 2, 3], [4, 5, 6, 7]]

_cache = {}


def _emit(ctx: ExitStack, tc: tile.TileContext, io: dict, t_steps: int):
    nc = tc.nc

    res = ctx.enter_context(tc.tile_pool(name="res", bufs=1))
    state = ctx.enter_context(tc.tile_pool(name="state", bufs=1))
    work = ctx.enter_context(tc.tile_pool(name="work", bufs=1))
    psum = ctx.enter_context(tc.tile_pool(name="psum", bufs=4, space="PSUM"))
    tpsum = ctx.enter_context(tc.tile_pool(name="tpsum", bufs=3, space="PSUM"))
    dram = ctx.enter_context(tc.tile_pool(name="dram", bufs=2, space="DRAM"))

    # ---- resident weights (fp16 hi/lo pairs) and constants ----
    wemb = [res.tile([65, G], F16, name=f"wemb{i}") for i in range(2)]
    whh0 = [res.tile([128, KT, G], F16, name=f"whh0{i}") for i in range(2)]
    wih1 = [res.tile([128, KT, G], F16, name=f"wih1{i}") for i in range(2)]
    whh1 = [res.tile([128, KT, G], F16, name=f"whh1{i}") for i in range(2)]
    wfc1 = [res.tile([128, 2, H], F16, name=f"wfc1{i}") for i in range(2)]
    wfc2 = [res.tile([128, KT, 64], F16, name=f"wfc2{i}") for i in range(2)]
    b1g = res.tile([128, G], F32)
    fc1b = res.tile([128, H], F32)
    lng = res.tile([128, H], F32)
    lnb = res.tile([128, H], F32)
    fc2b = res.tile([128, 64], F32)
    ident = res.tile([128, 128], F32)
    for i in range(2):
        sfx = ["hi", "lo"][i]
        nc.sync.dma_start(wemb[i][:], io[f"wemb_{sfx}"].ap())
        for t_, n_ in [(whh0, "whh0"), (wih1, "wih1"), (whh1, "whh1")]:
            nc.sync.dma_start(
                t_[i][:],
                io[f"{n_}_{sfx}"].ap().rearrange("(k p) n -> p k n", p=128))
        nc.sync.dma_start(
            wfc1[i][:],
            io[f"wfc1_{sfx}"].ap().rearrange("(k p) n -> p k n", p=128))
        nc.sync.dma_start(
            wfc2[i][:],
            io[f"wfc2_{sfx}"].ap().rearrange("(k p) n -> p k n", p=128))
    for t_, n_ in [(b1g, "b1g"), (fc1b, "fc1b"), (lng, "lng"),
                   (lnb, "lnb"), (fc2b, "fc2b"), (ident, "ident")]:
        nc.sync.dma_start(t_[:], io[n_].ap())
    eps_t = res.tile([128, 1], F32)
    nc.vector.memset(eps_t[:], LN_EPS)

    # ---- initial state ----
    yT = [state.tile([65, B2], F16, tag=f"yT{i}", name=f"yT_init{i}")
          for i in range(2)]
    h0T = [state.tile([128, KT, B2], F16, tag=f"h0T{i}", name=f"h0T_init{i}")
           for i in range(2)]
    h1T = [state.tile([128, KT, B2], F16, tag=f"h1T{i}", name=f"h1T_init{i}")
           for i in range(2)]
    c0 = state.tile([128, MC, HS], F32, tag="c0", bufs=2)
    c1 = state.tile([128, MC, HS], F32, tag="c1", bufs=2)
    for i in range(2):
        sfx = ["hi", "lo"][i]
        nc.sync.dma_start(yT[i][:], io[f"x0T_{sfx}"].ap())
        nc.sync.dma_start(
            h0T[i][:],
            io[f"h0T0_{sfx}"].ap().rearrange("(k p) b -> p k b", p=128))
        nc.sync.dma_start(
            h1T[i][:],
            io[f"h1T0_{sfx}"].ap().rearrange("(k p) b -> p k b", p=128))
    nc.sync.dma_start(
        c0[:], io["c00"].ap().rearrange("(mc p) h -> p mc h", p=128))
    nc.sync.dma_start(
        c1[:], io["c10"].ap().rearrange("(mc p) h -> p mc h", p=128))

    def mm3(p, lhsT, rhs, start, stop, n0, n1):
        """3-pass fp16 matmul accumulate: hi*hi + lo*hi + hi*lo."""
        nc.tensor.matmul(p, lhsT[0], rhs[0][:, n0:n1], start=start,
                         stop=False, skip_group_check=True)
        nc.tensor.matmul(p, lhsT[1], rhs[0][:, n0:n1], start=False,
                         stop=False, skip_group_check=True)
        nc.tensor.matmul(p, lhsT[0], rhs[1][:, n0:n1], start=False,
                         stop=stop, skip_group_check=True)

    def gates_hh(hT, w, tag):
        """Open a 4-bank psum group with the recurrent contribution."""
        ps = [[psum.tile([128, 512], F32, tag="p512",
                         name=f"{tag}_{mc}_{nk}") for nk in range(2)]
              for mc in range(2)]
        for mc in range(2):
            for nk in range(2):
                for k in range(KT):
                    mm3(ps[mc][nk][:],
                        [hT[i][:, k, mc * 128:(mc + 1) * 128] for i in range(2)],
                        [w[i][:, k, :] for i in range(2)],
                        start=(k == 0), stop=False,
                        n0=nk * 512, n1=(nk + 1) * 512)
        return ps

    def gates0_ih(ps, yT_pair):
        """Close the gates0 group with the folded-embedding input part
        (65 rows: 64 of y plus a ones-row carrying the bias)."""
        for mc in range(2):
            for nk in range(2):
                mm3(ps[mc][nk][:],
                    [yT_pair[i][:, mc * 128:(mc + 1) * 128] for i in range(2)],
                    wemb, start=False, stop=True,
                    n0=nk * 512, n1=(nk + 1) * 512)

    def gates1_ih(ps, h0T_new):
        for mc in range(2):
            for nk in range(2):
                for k in range(KT):
                    mm3(ps[mc][nk][:],
                        [h0T_new[i][:, k, mc * 128:(mc + 1) * 128]
                         for i in range(2)],
                        [wih1[i][:, k, :] for i in range(2)],
                        start=False, stop=(k == KT - 1),
                        n0=nk * 512, n1=(nk + 1) * 512)

    def evac_nonlin_cell(ps, bias, c_prev, layer):
        """psum -> gts (+bias), i|f sigmoid, g tanh, o sigmoid, cell update.
        Returns (h_sh [128,MC,HS] f32, c_new, g_t)."""
        g_t = work.tile([128, MC, G], F32, tag="gts", name=f"gts_l{layer}",
                        bufs=2)
        c_new = state.tile([128, MC, HS], F32, tag=f"c{layer}",
                           name=f"c{layer}n", bufs=2)
        h_sh = work.tile([128, MC, HS], F32, tag=f"h{layer}",
                         name=f"h{layer}sh", bufs=1)
        for mc in range(2):
            for nk in range(2):
                sl = slice(nk * 512, (nk + 1) * 512)
                if bias is None:
                    nc.vector.tensor_copy(g_t[:, mc, sl], ps[mc][nk][:])
                else:
                    nc.vector.tensor_add(g_t[:, mc, sl], ps[mc][nk][:],
                                         bias[:, sl])
            nc.scalar.activation(g_t[:, mc, 0:512], g_t[:, mc, 0:512],
                                 AF.Sigmoid)
            nc.scalar.activation(g_t[:, mc, 512:768], g_t[:, mc, 512:768],
                                 AF.Tanh)
            nc.scalar.activation(g_t[:, mc, 768:1024], g_t[:, mc, 768:1024],
                                 AF.Sigmoid)
            tmp = work.tile([128, HS], F32, tag="ctmp", name=f"ct{layer}{mc}",
                            bufs=2)
            tanhc = work.tile([128, HS], F32, tag="tanhc",
                              name=f"th{layer}{mc}", bufs=2)
            nc.vector.tensor_mul(tmp[:], g_t[:, mc, 256:512], c_prev[:, mc, :])
            nc.vector.tensor_mul(c_new[:, mc, :], g_t[:, mc, 0:256],
                                 g_t[:, mc, 512:768])
            nc.vector.tensor_add(c_new[:, mc, :], c_new[:, mc, :], tmp[:])
            nc.scalar.activation(tanhc[:], c_new[:, mc, :], AF.Tanh)
            nc.vector.tensor_mul(h_sh[:, mc, :], g_t[:, mc, 768:1024],
                                 tanhc[:])
        return h_sh, c_new, g_t

    def transpose_split(h_sh, tag):
        """[128,MC,HS] f32 -> transposed fp16 pair [128, 2(hb), B2]."""
        pair = [work.tile([128, 2, B2], F16, tag=f"{tag}{i}",
                          name=f"{tag}p{i}", bufs=1) for i in range(2)]
        for mc in range(2):
            for hb in range(2):
                tp = tpsum.tile([128, 128], F32, tag="tp", name=f"tp_{tag}")
                nc.tensor.transpose(tp[:], h_sh[:, mc, hb * 128:(hb + 1) * 128],
                                    ident[:])
                rt = work.tile([128, 128], F32, tag="rt", name=f"rt_{tag}",
                               bufs=2)
                bs = slice(mc * 128, (mc + 1) * 128)
                nc.scalar.activation(pair[0][:, hb, bs], tp[:], AF.Copy)
                nc.vector.tensor_sub(rt[:], tp[:], pair[0][:, hb, bs])
                nc.gpsimd.tensor_copy(pair[1][:, hb, bs], rt[:])
        return pair

    def ag_pair(pair, layer, t):
        """AllGather the transposed shard pair -> full [128, KT, B2] pair."""
        gin = dram.tile([2, 2, 128, B2], F16, tag=f"gin{layer}",
                        name=f"gin{layer}_{t}")
        gout = dram.tile([4, 2, 2, 128, B2], F16, tag=f"gout{layer}",
                         name=f"gout{layer}_{t}")
        for i in range(2):
            nc.sync.dma_start(
                gin[:, i].rearrange("hb p b -> p hb b"), pair[i][:])
        nc.gpsimd.collective_compute(
            "AllGather", ALU.bypass, replica_groups=GROUPS,
            ins=[gin.opt()], outs=[gout.opt()])
        full = [state.tile([128, KT, B2], F16, tag=f"h{layer}T{i}",
                           name=f"h{layer}Tn{i}") for i in range(2)]
        for i in range(2):
            nc.sync.dma_start(
                full[i][:],
                gout[:, :, i].rearrange("m hb p b -> p (m hb) b"))
        return full

    for t in range(t_steps):
        # ---- finish gates0(t): psum group was opened earlier ----
        if t == 0:
            g0 = gates_hh(h0T, whh0, "g0")
            gates0_ih(g0, yT)
        h0_sh, c0, _ = evac_nonlin_cell(g0, None, c0, 0)
        h0T_sh = transpose_split(h0_sh, "h0s")
        h0T = ag_pair(h0T_sh, 0, t)

        # ---- gates1(t): hh (ready) then ih (waits AG0) ----
        g1 = gates_hh(h1T, whh1, "g1")
        gates1_ih(g1, h0T)
        h1_sh, c1, gt1 = evac_nonlin_cell(g1, b1g, c1, 1)
        h1T_sh = transpose_split(h1_sh, "h1s")

        # ---- fc1 K-shard partial: z_part = h1T_sh.T @ wfc1 ----
        zps = [[psum.tile([128, 512], F32, tag="p512", name=f"z_{mc}_{nk}")
                for nk in range(2)] for mc in range(2)]
        for mc in range(2):
            for nk in range(2):
                for k in range(2):
                    mm3(zps[mc][nk][:],
                        [h1T_sh[i][:, k, mc * 128:(mc + 1) * 128]
                         for i in range(2)],
                        [wfc1[i][:, k, :] for i in range(2)],
                        start=(k == 0), stop=(k == 1),
                        n0=nk * 512, n1=(nk + 1) * 512)
        zp = work.tile([128, MC, H], F32, tag="z", name="zp", bufs=2)
        for mc in range(2):
            for nk in range(2):
                nc.vector.tensor_copy(zp[:, mc, nk * 512:(nk + 1) * 512],
                                      zps[mc][nk][:])

        # ---- AR(z), then AG(h1) ----
        rin = dram.tile([2, 128, H], F32, tag="rin", name=f"rin{t}")
        rout = dram.tile([2, 128, H], F32, tag="rout", name=f"rout{t}")
        nc.sync.dma_start(rin[:].rearrange("mc p n -> p mc n"), zp[:])
        nc.gpsimd.collective_compute(
            "AllReduce", ALU.add, replica_groups=GROUPS,
            ins=[rin.opt()], outs=[rout.opt()])
        h1T = ag_pair(h1T_sh, 1, t)

        # ---- open gates0(t+1) hh while AR(z) is in flight ----
        if t + 1 < t_steps:
            g0 = gates_hh(h0T, whh0, "g0")

        # ---- z reload, +fc1 bias, LayerNorm, ReLU (in place on z) ----
        z = work.tile([128, MC, H], F32, tag="z", name=f"z{t}", bufs=2)
        nc.sync.dma_start(z[:], rout[:].rearrange("mc p n -> p mc n"))
        for mc in range(2):
            s = work.tile([128, 1], F32, tag="s1", bufs=8, name=f"s{t}{mc}")
            mu = work.tile([128, 1], F32, tag="s1", bufs=8, name=f"mu{t}{mc}")
            negmu = work.tile([128, 1], F32, tag="s1", bufs=8,
                              name=f"nmu{t}{mc}")
            sqs = work.tile([128, 1], F32, tag="s1", bufs=8, name=f"sqs{t}{mc}")
            va = work.tile([128, 1], F32, tag="s1", bufs=8, name=f"va{t}{mc}")
            sv = work.tile([128, 1], F32, tag="s1", bufs=8, name=f"sv{t}{mc}")
            rstd = work.tile([128, 1], F32, tag="s1", bufs=8,
                             name=f"rs{t}{mc}")
            nc.vector.tensor_add(z[:, mc, :], z[:, mc, :], fc1b[:])
            nc.vector.reduce_sum(s[:], z[:, mc, :], axis=mybir.AxisListType.X)
            nc.vector.tensor_scalar_mul(negmu[:], s[:], -1.0 / H)
            nc.vector.tensor_scalar_mul(mu[:], s[:], 1.0 / H)
            # Square scratch: reuse the dead layer-1 gate tile
            nc.scalar.activation(gt1[:, mc, :], z[:, mc, :], AF.Square,
                                 bias=negmu[:], accum_out=sqs[:])
            nc.vector.tensor_scalar_mul(va[:], sqs[:], 1.0 / H)
            nc.scalar.activation(sv[:], va[:], AF.Sqrt, bias=eps_t[:])
            nc.vector.reciprocal(rstd[:], sv[:])
            nc.vector.tensor_scalar(z[:, mc, :], z[:, mc, :], mu[:], rstd[:],
                                    ALU.subtract, ALU.mult)
            nc.vector.tensor_mul(z[:, mc, :], z[:, mc, :], lng[:])
            nc.vector.tensor_add(z[:, mc, :], z[:, mc, :], lnb[:])
            nc.scalar.activation(z[:, mc, :], z[:, mc, :], AF.Relu)

        # ---- transpose relu -> fp16 pair [128, KT, B2] ----
        reluT = [work.tile([128, KT, B2], F16, tag=f"reluT{i}",
                           name=f"rT{i}_{t}", bufs=1) for i in range(2)]
        for mc in range(2):
            for hb in range(KT):
                tp = tpsum.tile([128, 128], F32, tag="tp", name=f"tp_r{t}")
                nc.tensor.transpose(
                    tp[:], z[:, mc, hb * 128:(hb + 1) * 128], ident[:])
                rt = work.tile([128, 128], F32, tag="rt", name=f"rt_r{t}",
                               bufs=2)
                bs = slice(mc * 128, (mc + 1) * 128)
                nc.scalar.activation(reluT[0][:, hb, bs], tp[:], AF.Copy)
                nc.vector.tensor_sub(rt[:], tp[:], reluT[0][:, hb, bs])
                nc.gpsimd.tensor_copy(reluT[1][:, hb, bs], rt[:])

        # ---- fc2 -> y [128, MC, 64], output DMA ----
        y = work.tile([128, MC, 64], F32, tag="y", name=f"y{t}", bufs=1)
        for mc in range(2):
            yp = tpsum.tile([128, 128], F32, tag="tp", name=f"yp{t}")
            for k in range(KT):
                mm3(yp[:, 0:64],
                    [reluT[i][:, k, mc * 128:(mc + 1) * 128] for i in range(2)],
                    [wfc2[i][:, k, :] for i in range(2)],
                    start=(k == 0), stop=(k == KT - 1), n0=0, n1=64)
            nc.vector.tensor_add(y[:, mc, :], yp[:, 0:64], fc2b[:, 0:64])
        nc.sync.dma_start(
            io["ys"].ap()[t].rearrange("(mc p) d -> p mc d", p=128), y[:])

        # ---- yT pair for next step's gates0 input part ----
        if t + 1 < t_steps:
            yTn = [state.tile([65, B2], F16, tag=f"yT{i}", name=f"yTn{i}_{t}")
                   for i in range(2)]
            yTf = work.tile([64, B2], F32, tag="yTf", name=f"yTf{t}", bufs=1)
            for mc in range(2):
                ytp = tpsum.tile([128, 128], F32, tag="tp", name=f"ytp{t}")
                nc.tensor.transpose(ytp[0:64, :], y[:, mc, :], ident[:])
                nc.vector.tensor_copy(yTf[:, mc * 128:(mc + 1) * 128],
                                      ytp[0:64, 0:128])
            nc.scalar.activation(yTn[0][0:64, :], yTf[:], AF.Copy)
            rty = work.tile([64, B2], F32, tag="rty", name=f"rty{t}", bufs=1)
            nc.vector.tensor_sub(rty[:], yTf[:], yTn[0][0:64, :])
            nc.gpsimd.tensor_copy(yTn[1][0:64, :], rty[:])
            nc.vector.memset(yTn[0][64:65, :], 1.0)
            nc.vector.memset(yTn[1][64:65, :], 0.0)
            yT = yTn
            gates0_ih(g0, yT)


def build(t_steps=T_FULL):
    if t_steps in _cache:
        return _cache[t_steps]
    nc = bacc.Bacc("TRN2", target_bir_lowering=False, debug=False,
                   num_devices=NC)
    io = {}
    inputs = [
        ("wemb_hi", (65, G), F16), ("wemb_lo", (65, G), F16),
        ("whh0_hi", (H, G), F16), ("whh0_lo", (H, G), F16),
        ("wih1_hi", (H, G), F16), ("wih1_lo", (H, G), F16),
        ("whh1_hi", (H, G), F16), ("whh1_lo", (H, G), F16),
        ("wfc1_hi", (HS, H), F16), ("wfc1_lo", (HS, H), F16),
        ("wfc2_hi", (H, 64), F16), ("wfc2_lo", (H, 64), F16),
        ("b1g", (128, G), F32),
        ("fc1b", (128, H), F32), ("lng", (128, H), F32),
        ("lnb", (128, H), F32), ("fc2b", (128, 64), F32),
        ("ident", (128, 128), F32),
        ("x0T_hi", (65, B2), F16), ("x0T_lo", (65, B2), F16),
        ("h0T0_hi", (H, B2), F16), ("h0T0_lo", (H, B2), F16),
        ("h1T0_hi", (H, B2), F16), ("h1T0_lo", (H, B2), F16),
        ("c00", (B2, HS), F32), ("c10", (B2, HS), F32),
    ]
    for name, shape, dt in inputs:
        io[name] = nc.dram_tensor(name, shape, dt, kind="ExternalInput")
    io["ys"] = nc.dram_tensor("ys", (t_steps, B2, D), F32,
                              kind="ExternalOutput")
    with tile.TileContext(nc) as tc:
        with ExitStack() as ctx:
            _emit(ctx, tc, io, t_steps)
    nc.compile()
    _cache[t_steps] = (nc, io)
    return nc, io


def _split16(x):
    hi = x.astype(np.float16)
    lo = (x - hi.astype(np.float64)).astype(np.float16)
    return np.ascontiguousarray(hi), np.ascontiguousarray(lo)


def make_in_maps(inputs):
    f64 = lambda k: np.asarray(inputs[k]).astype(np.float64)
    emb_W, emb_b = f64("emb_W"), f64("emb_b")
    W_ih0 = f64("W_ih0")
    Wemb = emb_W @ W_ih0                     # [64, 4096]
    b0 = f64("b_ih0") + f64("b_hh0") + emb_b @ W_ih0
    b1 = f64("b_ih1") + f64("b_hh1")
    W_hh0, W_ih1, W_hh1 = f64("W_hh0"), f64("W_ih1"), f64("W_hh1")
    fc1_W, fc2_W = f64("fc1_W"), f64("fc2_W")
    x0 = f64("x_0")
    hn, cn = f64("h_n"), f64("c_n")
    ones = np.ones((1, B2), np.float64)

    bc = lambda v: np.tile(v.astype(np.float32)[None, :], (128, 1))
    in_maps = []
    for c in range(NC):
        q, m = c // 4, c % 4
        bs = slice(q * B2, (q + 1) * B2)
        cols = np.concatenate(
            [np.arange(g * H + m * HS, g * H + (m + 1) * HS) for g in range(4)])
        m_ = {}
        wemb_aug = np.vstack([Wemb[:, cols], b0[cols][None, :]])  # [65, G]
        for name, w in [("wemb", wemb_aug), ("whh0", W_hh0[:, cols]),
                        ("wih1", W_ih1[:, cols]), ("whh1", W_hh1[:, cols]),
                        ("wfc1", fc1_W[m * HS:(m + 1) * HS, :]),
                        ("wfc2", fc2_W)]:
            m_[f"{name}_hi"], m_[f"{name}_lo"] = _split16(w)
        m_["b1g"] = bc(b1[cols])
        m_["fc1b"] = bc(f64("fc1_b"))
        m_["lng"] = bc(f64("ln_g"))
        m_["lnb"] = bc(f64("ln_b"))
        m_["fc2b"] = bc(f64("fc2_b"))
        m_["ident"] = np.eye(128, dtype=np.float32)
        m_["x0T_hi"], m_["x0T_lo"] = _split16(
            np.vstack([x0[bs].T, ones]))
        m_["h0T0_hi"], m_["h0T0_lo"] = _split16(hn[0][bs].T)
        m_["h1T0_hi"], m_["h1T0_lo"] = _split16(hn[1][bs].T)
        m_["c00"] = np.ascontiguousarray(
            cn[0][bs, m * HS:(m + 1) * HS].astype(np.float32))
        m_["c10"] = np.ascontiguousarray(
            cn[1][bs, m * HS:(m + 1) * HS].astype(np.float32))
        in_maps.append(m_)
    return in_maps


def kernel(**inputs):
    t_steps = int(inputs.get("forecast_window", T_FULL))
    nc, io = build(t_steps)
    in_maps = make_in_maps(inputs)
    r = bass_utils.run_bass_kernel_spmd(nc, in_maps, core_ids=list(range(NC)))
    out = np.empty((B, t_steps, D), np.float32)
    for q in range(2):
        ys = r.results[q * 4]["ys"]            # [t, B2, D]
        out[q * B2:(q + 1) * B2] = ys.transpose(1, 0, 2)
    return out
